# revision 53
# baseline (speedup 1.0000x reference)
"""Bass/Trainium2 kernel for nn_BiChannelAttention (single-query local-window attention).

Math (per batch b, head h, S=2049, window W=256, cutoff=S-W=1793):
  Positions before the cutoff get a -1e6 additive mask -> softmax weight exactly 0
  in fp32. Only the last W positions matter. The time_mask is a no-op (the
  reference's masked_fill chain shifts every score by the same -1e6).

  Window rows X [W=256, 128] (last 255 cache rows + content row):
    kq   = mfold_h^T cnt_h,  mfold = (256/sqrt(128)) Wq_h Wk_h^T  (host-folded;
           the x256 scale keeps kq in fp8 normal range, undone inside exp)
    sc   = X kq                                          [256]  (= 256*score)
    a    = exp(sc/256)          (scores are O(0.05); no max-subtraction needed)
    xa   = X'^T a          X' = e^bias * X  (positional bias folded host-side)
    den  = expb^T a        expb = e^bias stationary (ones-matmul trick)
    out  = Wv_h^T xa / den + cnt_h

Precision: everything on the matmul path is fp8e4m3 with fp32 PSUM accumulation
(scores are tiny so softmax is insensitive; the output is dominated by the f32
residual, and fp8 errors average across the 256-wide near-uniform attention).
Measured rel err ~1.1e-3 vs the 2e-2 gate.

PE work per head: 1 kq matmul + 16 score matvecs (stationary xt tile [d,s]) +
2 den matmuls + 16 accumulating xa matvecs + 1 output projection, all fp8
(ld/matmul pairs pipeline at ~40 ns). No on-chip transposes: the host ships the
window in BOTH layouts ([d,s] for scores, [s,t,d] for xa), ~1.1 MB/core total.

The kernel is DMA-arrival-bound: queues share ~200 GB/s aggregate, so the
transfers are ordered/split so each consumer's tensor lands just in time
(xt0 -> xt1 -> x0 -> x1), and all non-PE ops are spread across vector/scalar/
gpsimd with the scheduler free to fill PE bubbles.

Sharding: tensor-parallel over heads, 2 heads per core x 8 cores.
"""

import sys
import numpy as np
import ml_dtypes

for _p in ("/opt/trn_rl_repo", "/root/.axon_site/_ro/trn_rl_repo"):
    if _p not in sys.path:
        sys.path.insert(0, _p)

import concourse.bass as bass
import concourse.bacc as bacc
import concourse.mybir as mybir
from concourse.tile import TileContext
from concourse.bass_utils import run_bass_kernel_spmd

F32 = mybir.dt.float32
BF16 = mybir.dt.bfloat16
F8 = mybir.dt.float8e4
NP_F8 = ml_dtypes.float8_e4m3
NP_BF16 = ml_dtypes.bfloat16

P = 128          # partitions / head_dim
B = 8            # batch
H = 16           # heads total
HPC = 2          # heads per core
NCORES = 8
T = 2048
S = T + 1
W = 256          # local attention window
NT = 2           # s-tiles per window
CUTOFF = S - W   # 1793
KSCALE = 256.0   # fp8 dynamic-range scale folded into wkt (and undone in exp)

_NC_CACHE = {}


def _build_nc():
    nc = bacc.Bacc(None, target_bir_lowering=False, debug=False)
    # xt: [j, d, b*W+s] fp8 -- scores stationary tiles [d, s]
    xt_in = nc.declare_dram_parameter("xt", [HPC, P, B * W], F8, isOutput=False)
    # x: [j, s_lo, b, t, d] fp8 -- xa stationary tiles [s_lo, d]
    x_in = nc.declare_dram_parameter("x", [HPC, P, B, NT, P], F8, isOutput=False)
    # fp8 consts: mfold0|wv0|mfold1|wv1|cnt(j*8+b)|expb(t=0,1);
    # mfold = 256/sqrt(128)*Wq@Wk^T, expb = e^bias compact [p, t]
    c8_in = nc.declare_dram_parameter("c8", [P, 4 * P + HPC * B + NT], F8, isOutput=False)
    # f32 consts: residual content [p, j*8+b]
    cf_in = nc.declare_dram_parameter("cf", [P, HPC * B], F32, isOutput=False)
    out_t = nc.declare_dram_parameter("out", [P, HPC * B], F32, isOutput=True)

    with TileContext(nc) as tc:
        with (
            tc.tile_pool(name="const", bufs=1) as cpool,
            tc.tile_pool(name="data", bufs=2) as dpool,
            tc.tile_pool(name="sm", bufs=2) as spool,
            tc.tile_pool(name="ps_a", bufs=1, space="PSUM") as psa,
            tc.tile_pool(name="ps_sc", bufs=2, space="PSUM") as pssc,
            tc.tile_pool(name="ps_b", bufs=1, space="PSUM") as psb,
        ):
            # big window tensors split in halves across queues, priority order:
            # per-queue DMA tops out near ~100 GB/s (independent of line
            # size), aggregate ~210-235 GB/s, so pair halves per tensor.
            # sync carries the small consts first, then the x0 halves.
            xt_sb, x_sb = [], []
            xt_sb.append(dpool.tile([P, B * W], F8, tag="xt", name="xt0"))
            x_sb.append(dpool.tile([P, B, NT, P], F8, tag="x", name="x0"))
            xt_sb.append(dpool.tile([P, B * W], F8, tag="xt", name="xt1"))
            x_sb.append(dpool.tile([P, B, NT, P], F8, tag="x", name="x1"))
            HB = B // 2
            HW = HB * W
            # arrival order is the only real dial (the queues share ~200 GB/s
            # aggregate): xt halves paired across the two data queues so xt0
            # lands first and xt1 second; x1 takes the data queues' third
            # slot; x0 rides the sync queue behind the small consts
            for j in range(HPC):
                nc.gpsimd.dma_start(out=xt_sb[j][:, 0:HW], in_=xt_in[j, :, 0:HW])
                nc.scalar.dma_start(out=xt_sb[j][:, HW:], in_=xt_in[j, :, HW:])
            nc.gpsimd.dma_start(out=x_sb[1][:, 0:HB], in_=x_in[1, :, 0:HB])
            nc.scalar.dma_start(out=x_sb[1][:, HB:], in_=x_in[1, :, HB:])

            c8 = cpool.tile([P, 4 * P + HPC * B + NT], F8, tag="c8")
            nc.sync.dma_start(out=c8[:, :], in_=c8_in[:, :])
            cf = cpool.tile([P, HPC * B], F32, tag="cf")
            nc.sync.dma_start(out=cf[:, :], in_=cf_in[:, :])
            nc.sync.dma_start(out=x_sb[0][:, :, :, :], in_=x_in[0, :, :, :, :])

            # expand the compact e^bias columns to a [p, t, 128] stationary
            # on-chip (no DMA dependency for the denominator matmuls)
            expb = cpool.tile([P, NT, P], F8, tag="expb")
            nc.gpsimd.tensor_copy(
                expb[:, :, :],
                c8[:, 4 * P + HPC * B:].unsqueeze(2).broadcast_to([P, NT, P]),
            )

            def sc_sweep(j, kq8):
                sc_ps = pssc.tile([P, B, NT], F32, tag="sc", name=f"sc{j}")
                att_j = spool.tile([P, B, NT], F8, tag="att", name=f"att{j}")
                # the scalar queue dispatches ~0.7us before gpsimd's, so the
                # b4-7 half of xt lands first -- sweep it first (subtile deps
                # let those matvecs run while the b0-3 half still transfers)
                for b in (4, 5, 6, 7, 0, 1, 2, 3):
                    for t in range(NT):
                        nc.tensor.matmul(
                            sc_ps[:, b, t:t + 1],
                            xt_sb[j][:, b * W + t * P: b * W + (t + 1) * P],
                            kq8[:, b:b + 1],
                            start=True, stop=True,
                        )
                nc.scalar.activation(att_j[:, :, :], sc_ps[:, :, :],
                                     mybir.ActivationFunctionType.Exp,
                                     scale=1.0 / KSCALE)
                return att_j

            def den_mms(j, att_j):
                for t in range(NT):
                    nc.tensor.matmul(b_ps[:, j * B:(j + 1) * B],
                                     expb[:, t, :], att_j[:, :, t],
                                     start=(t == 0), stop=(t == NT - 1))

            def xa_sweep(j, att_j):
                for b in range(B):
                    for t in range(NT):
                        nc.tensor.matmul(
                            b_ps[:, 2 * B + j * B + b:2 * B + j * B + b + 1],
                            x_sb[j][:, b, t, :],
                            att_j[:, b, t:t + 1],
                            start=(t == 0), stop=(t == NT - 1),
                        )

            mfold = [c8[:, (2 * j) * P:(2 * j + 1) * P] for j in range(HPC)]
            wv = [c8[:, (2 * j + 1) * P:(2 * j + 2) * P] for j in range(HPC)]
            cnt_8 = c8[:, 4 * P:4 * P + HPC * B]
            cnt_f = cf

            # PE order: kq0, kq1, sc0, den0, sc1, xa0, den1, xa1, wv0, wv1 --
            # den0 fills the sc0->sc1 bubble while xt1 transfers; xa0 runs as
            # soon as x0 + att0 are in; den1/xa1 follow att1/x1. Per-head
            # vector ops are merged into single [128, 16] ops on shared PSUM
            # tiles (dens/xas/os as column groups) -- fewer cross-engine
            # semaphores, which the epilogue clears serially at ~115ns each.
            kq_ps = psa.tile([P, HPC * B], F32, tag="kq")
            for j in range(HPC):
                nc.tensor.matmul(kq_ps[:, j * B:(j + 1) * B], mfold[j],
                                 cnt_8[:, j * B:(j + 1) * B],
                                 start=True, stop=True)
            kq8 = spool.tile([P, HPC * B], F8, tag="kq8")
            nc.vector.tensor_copy(kq8[:, :], kq_ps[:, :])

            # shared phase-B PSUM: den j0|den j1|xa j0|xa j1|o j0|o j1
            b_ps = psb.tile([P, 6 * B], F32, tag="bps")
            fin = spool.tile([P, HPC * B], F32, tag="fin", bufs=1)

            att0 = sc_sweep(0, kq8[:, 0:B])
            den_mms(0, att0)
            att1 = sc_sweep(1, kq8[:, B:2 * B])
            xa_sweep(0, att0)
            den_mms(1, att1)
            rec = spool.tile([P, HPC * B], F32, tag="rec")
            nc.vector.reciprocal(rec[:, :], b_ps[:, 0:2 * B])
            xa_sweep(1, att1)
            xa8 = spool.tile([P, HPC * B], F8, tag="xab")
            nc.vector.tensor_copy(xa8[:, :], b_ps[:, 2 * B:4 * B])

            for j in range(HPC):
                nc.tensor.matmul(b_ps[:, 4 * B + j * B:4 * B + (j + 1) * B],
                                 wv[j], xa8[:, j * B:(j + 1) * B],
                                 start=True, stop=True)
            t1 = spool.tile([P, HPC * B], F32, tag="t1")
            nc.vector.tensor_mul(t1[:, :], b_ps[:, 4 * B:6 * B], rec[:, :])
            nc.vector.tensor_add(fin[:, :], t1[:, :], cnt_f[:, :])
            # single output store; the sync engine sits idle at the end so its
            # dispatch waits in parallel with the final add
            nc.sync.dma_start(out=out_t[:, :], in_=fin[:, :])
    nc.finalize()
    return nc


def _get_nc():
    if "nc" not in _NC_CACHE:
        _NC_CACHE["nc"] = _build_nc()
    return _NC_CACHE["nc"]


def _pos_bias_f32():
    """t5_position_bucket exactly as the reference computes it, sliced to the
    window."""
    if "pos" not in _NC_CACHE:
        import jax.numpy as jnp
        NUM_BUCKETS, MAX_DISTANCE = 32, 128
        n = (S - 1) - jnp.arange(S)
        max_exact = NUM_BUCKETS // 2
        is_small = n < max_exact
        large = max_exact + (
            jnp.log(jnp.maximum(n, 1).astype(jnp.float32) / max_exact)
            / np.log(MAX_DISTANCE / max_exact)
            * (NUM_BUCKETS - max_exact)
        ).astype(jnp.int32)
        large = jnp.minimum(large, NUM_BUCKETS - 1)
        pos = jnp.where(is_small, n, large).astype(jnp.float32)
        _NC_CACHE["pos"] = np.asarray(pos)[CUTOFF:]  # [W]
    return _NC_CACHE["pos"]


def kernel(**inputs) -> np.ndarray:
    t = int(np.asarray(inputs["t"]))
    assert t == T, f"kernel hardcoded for t={T}, got {t}"
    content_t = np.ascontiguousarray(np.asarray(inputs["content_t"], dtype=np.float32))
    cache = np.asarray(inputs["cache"], dtype=np.float32)
    Wq = np.asarray(inputs["Wq"], dtype=np.float32)
    Wk = np.asarray(inputs["Wk"], dtype=np.float32)
    Wv = np.asarray(inputs["Wv"], dtype=np.float32)
    pos_param = np.float32(np.asarray(inputs["pos_param"]))

    pos = _pos_bias_f32()                                   # [W]
    posb = (-pos_param * pos).astype(np.float32)            # [W]
    ebias = np.exp(posb).astype(np.float32)                 # [W] e^bias
    cnt_h = content_t.reshape(B, H, P)                      # [B, H, 128]
    # full window per (b, h): last 255 cache rows + content row
    win = np.empty((B, H, W, P), np.float32)
    win[:, :, : W - 1, :] = cache[:, CUTOFF:T, :].reshape(B, W - 1, H, P).transpose(0, 2, 1, 3)
    win[:, :, W - 1, :] = cnt_h
    win8 = win.astype(NP_F8)                                # [B, H, 256, 128] fp8 (scores)
    win8f = (win * ebias[None, None, :, None]).astype(NP_F8)  # e^bias folded (values)

    # fold q and k projections + scaling into one matrix:
    #   kq = (KSCALE/sqrt(128)) * Wk_h (Wq_h^T cnt) = mfold_h^T cnt,
    #   mfold_h = (KSCALE/sqrt(128)) * Wq_h @ Wk_h^T
    mfold = np.einsum("hde,hfe->hdf", Wq, Wk) * np.float32(KSCALE / np.sqrt(128.0))

    in_maps = []
    for c in range(NCORES):
        h0 = HPC * c
        xt_host = np.empty((HPC, P, B * W), NP_F8)
        x_host = np.empty((HPC, P, B, NT, P), NP_F8)
        for j in range(HPC):
            xt_host[j] = win8[:, h0 + j].transpose(2, 0, 1).reshape(P, B * W)
            x_host[j] = win8f[:, h0 + j].reshape(B, NT, P, P).transpose(2, 0, 1, 3)
        blocks = []
        for j in range(HPC):
            blocks += [mfold[h0 + j], Wv[h0 + j]]
        cntT = cnt_h[:, h0:h0 + HPC, :].transpose(2, 1, 0).reshape(P, HPC * B)
        blocks.append(cntT)
        blocks.append(ebias.reshape(NT, P).transpose(1, 0))  # expb compact [p, t]
        c8_host = np.concatenate(blocks, axis=1).astype(NP_F8)
        in_maps.append({
            "xt": xt_host, "x": x_host, "c8": c8_host,
            "cf": np.ascontiguousarray(cntT).astype(np.float32),
        })

    nc = _get_nc()
    res = run_bass_kernel_spmd(nc, in_maps, list(range(NCORES)), **_RUN_KWARGS)
    _NC_CACHE["last_results"] = res
    outs = np.stack([np.asarray(res.results[c]["out"]) for c in range(NCORES)])
    # outs: [core, d, j*8+b] -> out_full[b, (2c+j)*128 + d]
    out_full = outs.reshape(NCORES, P, HPC, B).transpose(3, 0, 2, 1).reshape(B, H * P)
    return out_full.astype(np.float32)


_RUN_KWARGS = {}  # test harness may set {"trace": True, "tmpdir": ...}
